# revision 2
# baseline (speedup 1.0000x reference)
# Trainium2 Bass kernel for nn_AttentionCombiner (self-attention where Q=K=V
# come from concat(output1, output2), followed by an output projection).
#
# Sharding: 8 cores = 4 batches x 2 head-groups (4 heads each). Each core
# computes its 4 heads' attention over ALL 2048 queries of one batch, plus the
# fc_out partial contraction over its heads' 512 features. The two partials
# per batch are summed on the host during unshard (plus bias), so the device
# program has NO collectives at all: with the symmetric-energy trick the
# free-axis accum over all 2048 q columns IS the full softmax row-sum.
#
# Key algebra (per batch, per head; X = combined features [2048, 128]):
#   E = X X^T (symmetric), S = exp(E/sqrt(d))
#   out^T[d, q] = sum_l X[l, d] S[l, q]          (unnormalized)
#   partial[q, o] = sum_{h in my 4} (1/r_h[q]) sum_d out^T_h[d, q] W[h*128+d, o]
#   r_h[q] = sum_l S_h[l, q] = sum_q' S_h[q, q']  (symmetry: row sums equal
#                                                  column sums; each core sees
#                                                  all 2048 q' so no comms)
#
# Loop: per head, 2 query-chunks of 1024; per chunk, 16 l-blocks of 128.
# Row-sum partials land in racc[:, c*16+i] via a cheap DVE junk pass
# (accum_out); per-head finish work (racc pair-add, reciprocal, 16 fc
# matmul+normalize+combine steps) is deferred and paced into the next head's
# attention stream.

import numpy as np
import ml_dtypes

N, S, D_IN, HEADS = 4, 2048, 512, 8
HEAD_DIM = 128          # 2*D_IN // HEADS
DF = 2 * D_IN           # 1024 combined features
HL = 4                  # heads per core (head-group)
DG = HL * HEAD_DIM      # 512 features per core
NB = S // 128           # 16 l-blocks
NC_CHUNK = 2            # query chunks per head
QC = S // NC_CHUNK      # 1024 queries per chunk
QB = S // 128           # 16 q-blocks
ISQ = 1.0 / float(np.sqrt(np.float32(HEAD_DIM)))

_CACHED_NC = None


def _build_nc(no_collective=False, fc_split=False, copy_eng="D", slack=True):
    import concourse.mybir as mybir
    import concourse.tile as tile
    from concourse import bacc
    from concourse.bass import ts

    f32 = mybir.dt.float32
    bf16 = mybir.dt.bfloat16
    Exp = mybir.ActivationFunctionType.Exp
    mult = mybir.AluOpType.mult
    add = mybir.AluOpType.add

    nc = bacc.Bacc("TRN2", target_bir_lowering=False, debug=False, num_devices=8)

    xth = nc.dram_tensor("xth", [DG, S], bf16, kind="ExternalInput")   # X^T my-head rows
    xb = nc.dram_tensor("xb", [S, DG], bf16, kind="ExternalInput")     # X my-head cols
    w = nc.dram_tensor("w", [DG, D_IN], bf16, kind="ExternalInput")    # W_out my rows
    bias = nc.dram_tensor("bias", [128, D_IN], f32, kind="ExternalInput")
    out = nc.dram_tensor("out", [S, D_IN], f32, kind="ExternalOutput")

    with tile.TileContext(nc) as tc:
        with (
            tc.tile_pool(name="persist", bufs=1) as pers,
            tc.tile_pool(name="spool", bufs=6 if slack else 4) as spool,
            tc.tile_pool(name="outp", bufs=4 if slack else 3) as outp,
            tc.tile_pool(name="rpool", bufs=4) as rpool,
            tc.tile_pool(name="psE", bufs=2, space="PSUM") as psE,
            tc.tile_pool(name="psO", bufs=1, space="PSUM") as psO,
            tc.tile_pool(name="psFC", bufs=2, space="PSUM") as psFC,
        ):
            # ---- persistent SBUF data ----
            xth_sb = pers.tile([128, HL, S], bf16, name="xth_sb")
            xb_sb = pers.tile([128, NB, DG], bf16, name="xb_sb")
            w_sb = pers.tile([128, HL, D_IN], bf16, name="w_sb")
            bias_sb = pers.tile([128, D_IN], f32, name="bias_sb")

            xth_r = xth.ap().rearrange("(h p) s -> p h s", p=128)
            xb_r = xb.ap().rearrange("(i p) d -> p i d", p=128)
            w_r = w.ap().rearrange("(h p) o -> p h o", p=128)

            # Front-load head 0 so compute starts promptly: the first piece
            # covers MM1(g=0)'s lhsT (cols 0:128) and rhs (cols 0:512).
            nc.sync.dma_start(xth_sb[:, 0, 0:512], xth_r[:, 0, 0:512])
            nc.sync.dma_start(xth_sb[:, 0, 512:S], xth_r[:, 0, 512:S])
            for i in range(NB):
                nc.sync.dma_start(xb_sb[:, i, :], xb_r[:, i, :])
            for h in range(1, HL):
                nc.sync.dma_start(xth_sb[:, h, :], xth_r[:, h, :])
            for h in range(HL):
                nc.sync.dma_start(w_sb[:, h, :], w_r[:, h, :])
            nc.sync.dma_start(bias_sb[:], bias.ap())

            # fc accumulators, persist across heads
            accs = [pers.tile([128, D_IN], f32, name=f"acc{j}")
                    for j in range(QB)]
            junk_d = pers.tile([128, QC], bf16, name="junk_d")
            junk_p = pers.tile([128, QC], bf16, name="junk_p")

            # deferred per-head finish work, paced into the next head's stream
            pending = []

            def emit_step():
                if pending:
                    pending.pop(0)()

            def emit_finish_head(h, racc, outTs):
                rmy = rpool.tile([128, QB], f32, tag="rmy", name="rmy")
                recip = rpool.tile([128, QB], f32, tag="recip", name="recip")

                def recip_step():
                    # full row-sums = chunk-0 partial + chunk-1 partial
                    nc.vector.scalar_tensor_tensor(
                        rmy[:], racc[:, 0:QB], 1.0, racc[:, QB : 2 * QB],
                        mult, add)
                    nc.vector.reciprocal(recip[:], rmy[:])

                pending.append(recip_step)

                for j in range(QB):
                    def step(h=h, j=j, outTs=outTs, recip=recip):
                        c, jj = j // (QC // 128), j % (QC // 128)
                        pfc = psFC.tile([128, D_IN], f32, tag="pfc", name="pfc")
                        nc.tensor.matmul(pfc[:], outTs[c][:, ts(jj, 128)],
                                         w_sb[:, h, :], start=True, stop=True)
                        # normalize by 1/r and combine across heads (DVE only:
                        # GPSIMD/Pool cannot read PSUM on TRN2)
                        eng = nc.vector
                        if h == 0:
                            eng.scalar_tensor_tensor(
                                accs[j][:], pfc[:], recip[:, j : j + 1],
                                bias_sb[:], mult, add)
                        else:
                            eng.scalar_tensor_tensor(
                                accs[j][:], pfc[:], recip[:, j : j + 1],
                                accs[j][:], mult, add)
                        if h == HL - 1:
                            nc.sync.dma_start(out.ap()[ts(j, 128), :],
                                              accs[j][:])
                    pending.append(step)

            # MM1 prefetch over a flattened (head, chunk, lblock) index so the
            # ACT exp stream never idles at head/chunk boundaries.
            pse_tiles = {}

            def mm1(g):
                h, c, i = g // (NC_CHUNK * NB), (g // NB) % NC_CHUNK, g % NB
                pse = psE.tile([128, QC], f32, tag="pse", name="pse")
                lhs1 = xth_sb[:, h, ts(i, 128)]
                nc.tensor.matmul(pse[:, 0:512], lhs1,
                                 xth_sb[:, h, c * QC : c * QC + 512],
                                 start=True, stop=True)
                nc.tensor.matmul(pse[:, 512:1024], lhs1,
                                 xth_sb[:, h, c * QC + 512 : c * QC + 1024],
                                 start=True, stop=True)
                pse_tiles[g] = pse

            NG = HL * NC_CHUNK * NB  # 128 tiles
            mm1(0)
            mm1(1)
            pso = None
            racc = None
            outTs = None
            for g in range(NG):
                h, c, i = g // (NC_CHUNK * NB), (g // NB) % NC_CHUNK, g % NB
                if c == 0 and i == 0:
                    racc = rpool.tile([128, NC_CHUNK * QB], f32, tag="racc",
                                      name="racc")
                    outTs = {}
                if i == 0:
                    pso = psO.tile([128, QC], f32, tag="pso", name="pso")
                pse = pse_tiles.pop(g)
                s_i = spool.tile([128, QC], bf16, tag="s", name="s_i")
                nc.scalar.activation(s_i[:], pse[:], Exp, bias=0.0, scale=ISQ)
                # row-sum partial via cheap DVE pass (4x mode: bf16, SBUF)
                nc.vector.tensor_scalar(
                    junk_d[:], s_i[:], 1.0, 0.0, mult, add,
                    accum_out=racc[:, c * QB + i : c * QB + i + 1])
                if g + 2 < NG:
                    mm1(g + 2)
                lhs2 = xb_sb[:, i, ts(h, 128)]
                nc.tensor.matmul(pso[:, 0:512], lhs2, s_i[:, 0:512],
                                 start=(i == 0), stop=(i == NB - 1))
                nc.tensor.matmul(pso[:, 512:1024], lhs2, s_i[:, 512:1024],
                                 start=(i == 0), stop=(i == NB - 1))
                # pace deferred finish work: ~1 step per 2 tiles; the last
                # head drains at full rate so little is left at the end
                if i % 2 == 1 or h == HL - 1:
                    emit_step()

                if i == NB - 1:
                    outT = outp.tile([128, QC], bf16, tag="outT", name="outT")
                    if h == HL - 1:
                        # tail shadow: ACT is idle once attention ends, and
                        # this keeps the DVE free for the fc drain
                        nc.scalar.copy(outT[:], pso[:])
                    else:
                        nc.vector.tensor_copy(outT[:], pso[:])
                    outTs[c] = outT
                    if c == NC_CHUNK - 1:
                        emit_finish_head(h, racc, outTs)

            while pending:
                emit_step()

    nc.compile()
    return nc


def _get_nc():
    global _CACHED_NC
    if _CACHED_NC is None:
        _CACHED_NC = _build_nc()
    return _CACHED_NC


def _make_in_maps(output1, output2, W_out, b_out):
    bf = ml_dtypes.bfloat16
    X = np.concatenate([np.asarray(output1), np.asarray(output2)], axis=2)
    Xb = X.astype(bf)                                   # [N, S, DF]
    Wb = np.asarray(W_out).astype(bf)                   # [DF, D_IN]
    bias_full = np.ascontiguousarray(
        np.broadcast_to(np.asarray(b_out).astype(np.float32), (128, D_IN)))
    zeros_bias = np.zeros((128, D_IN), np.float32)

    in_maps = []
    for c in range(8):
        n, hg = c // 2, c % 2
        Xn = Xb[n]                                      # [S, DF]
        in_maps.append({
            "xth": np.ascontiguousarray(Xn.T[hg * DG : (hg + 1) * DG, :]),
            "xb": np.ascontiguousarray(Xn[:, hg * DG : (hg + 1) * DG]),
            "w": np.ascontiguousarray(Wb[hg * DG : (hg + 1) * DG, :]),
            "bias": bias_full if hg == 0 else zeros_bias,
        })
    return in_maps


def kernel(output1, output2, W_out, b_out):
    from concourse.bass_utils import run_bass_kernel_spmd

    in_maps = _make_in_maps(output1, output2, W_out, b_out)
    nc = _get_nc()
    res = run_bass_kernel_spmd(nc, in_maps, core_ids=list(range(8)))

    full = np.empty((N, S, D_IN), np.float32)
    for n in range(N):
        full[n] = res.results[2 * n]["out"] + res.results[2 * n + 1]["out"]
    return full
